# revision 64
# baseline (speedup 1.0000x reference)
"""GCLSTM Trainium2 Bass kernel.

Data-parallel over batch B=64 across 8 NeuronCores (8 batches/core).

The 168-step LSTM recurrence is latency-bound: total time ~= 169 x the
serial per-step chain PE -> Act(sigmoid) -> DVE(cell) -> Act(sig 2c) ->
DVE(m) -> PE. Everything else (temporal stats, GraphConv, Conv1D) is
emitted as small work units sprinkled between LSTM steps so it executes
inside the chain's idle engine time instead of serializing in front.

LSTM structure per step (transposed layout, units on partitions, all
four gates through ONE sigmoid):
  - gate order [i, f, o, g]; g-chunk weights pre-scaled x2 so
    tanh(zg) = 2*sig(2*zg) - 1 comes from the same sigmoid instr
  - h never materializes: h = 2*m - o with m = o*sig(2c); PE applies
    rkO = -rk to o right after the sigmoid (off the critical path) and
    rkM = 2*rk to m once the cell update lands
  - cell update: v = sig(f)*c;  a = (2*sg-1)*i  (one custom-DVE op);
    c = v + a (affine_then_add); m = sig(2c)*o (affine_mul_reduce)
"""

import os
import numpy as np
from contextlib import ExitStack

import concourse.bass as bass
import concourse.tile as tile
from concourse import bacc, mybir
from concourse.bass_utils import run_bass_kernel_spmd

F32 = mybir.dt.float32
F32R = mybir.dt.float32r
N_CORES = 8
B, H, N, F, P = 64, 168, 512, 8, 24
BL = B // N_CORES          # 8 batches per core
HH = H // 2                # 84
T = H                      # 168 time steps
U = 128                    # LSTM units
GB = BL                    # all 8 batches in one LSTM group
NCH = N // 128             # 4 node chunks
NBC = BL * NCH             # 32 (b, nchunk) tiles

_K168 = 1.0 / 168.0
_K84 = 1.0 / 84.0
_KSLOPE = 1.0 / float(168 * (168 * 168 - 1) // 12)  # 1/sum(tc^2)

_CACHE = {}


def _emit_kernel(nc, tc, ctx, dbg=None):
    d = {k: nc.dram_tensor(k, shp, F32, kind="ExternalInput").ap()
         for k, shp in [
             ("adjT", [N, N]),
             ("tc_bc", [128, H]), ("I128", [128, 128]), ("ones_row", [1, 128]),
             ("w1", [7, 32]), ("b1row", [1, 32]), ("w2", [32, 16]),
             ("b2row", [1, 16]),
             ("w1c", [3, N, 4]), ("b1c2", [4, 1]), ("w2ch", [4, 3, 4]),
             ("b2c", [4, 1]),
             ("b1p", [128, 4]),
             ("rkM1", [U, 512]), ("rkO1", [U, 512]),
             ("kM2", [U, 512]), ("kO2", [U, 512]),
             ("rkM2", [U, 512]), ("rkO2", [U, 512]),
             ("b2p4", [4, 128]), ("sel4", [4, 4 * GB]),
             ("Whead", [16, 4, P]), ("Wlstm", [U, P]), ("b_out_row", [1, P]),
         ]}
    for k, shp in [("seqT", [F, T * BL]), ("k1p", [F, 512])]:
        d[k] = nc.dram_tensor(k, shp, F32R, kind="ExternalInput").ap()
    d["x0t"] = nc.dram_tensor("x0t", [BL, N, H], F32,
                              kind="ExternalInput").ap()
    out = nc.dram_tensor("out", [BL, P], F32, kind="ExternalOutput").ap()

    # ---------------- pools (PSUM: 2 + 2 + 2 = 6 banks) ----------------
    consts = ctx.enter_context(tc.tile_pool(name="consts", bufs=1))
    xpool = ctx.enter_context(tc.tile_pool(name="xpool", bufs=3))
    scr = ctx.enter_context(tc.tile_pool(name="scr", bufs=2))
    stats = ctx.enter_context(tc.tile_pool(name="stats", bufs=1))
    gcn = ctx.enter_context(tc.tile_pool(name="gcn", bufs=1))
    lstm = ctx.enter_context(tc.tile_pool(name="lstm", bufs=1))
    zpool = ctx.enter_context(tc.tile_pool(name="zpool", bufs=3))
    ps_zx = ctx.enter_context(tc.tile_pool(name="ps_zx", bufs=2, space="PSUM"))
    ps_a = ctx.enter_context(tc.tile_pool(name="ps_a", bufs=2, space="PSUM"))
    ps_z = ctx.enter_context(tc.tile_pool(name="ps_z", bufs=2, space="PSUM"))

    def load(pool, name, shape=None):
        t = pool.tile(shape or list(d[name].shape), d[name].dtype,
                      tag=name, name=name)
        nc.sync.dma_start(t[:], d[name][:])
        return t

    # ---------------- resident constants (LSTM path first) ----------------
    seqT = load(consts, "seqT")
    k1p = load(consts, "k1p")
    b1p = load(consts, "b1p")
    I128 = load(consts, "I128")
    rkM1 = load(consts, "rkM1")
    rkO1 = load(consts, "rkO1")
    kM2 = load(consts, "kM2")
    kO2 = load(consts, "kO2")
    rkM2 = load(consts, "rkM2")
    rkO2 = load(consts, "rkO2")
    b2p4 = load(consts, "b2p4")
    sel4 = load(consts, "sel4")
    adjT = consts.tile([128, NCH * N], F32, tag="adjT")
    for mc in range(NCH):
        nc.sync.dma_start(adjT[:, mc * N:(mc + 1) * N],
                          d["adjT"][mc * 128:(mc + 1) * 128, :])
    tcb = load(consts, "tc_bc")
    onesr = load(consts, "ones_row")
    w1 = load(consts, "w1")
    b1row = load(consts, "b1row")
    w2 = load(consts, "w2")
    b2row = load(consts, "b2row")
    b1c2 = load(consts, "b1c2")
    w2ch = load(consts, "w2ch")
    b2c = load(consts, "b2c")
    Whead = load(consts, "Whead")
    Wlstm = load(consts, "Wlstm")
    b_out_row = load(consts, "b_out_row")
    zer168 = consts.tile([128, H], F32, tag="zer168")
    nc.gpsimd.memset(zer168[:], 0.0)
    wc1 = consts.tile([128, 3, NCH, 4], F32, tag="wc1sb")
    for dd in range(3):
        for nk in range(NCH):
            nc.sync.dma_start(wc1[:, dd, nk, :],
                              d["w1c"][dd, nk * 128:(nk + 1) * 128, :])

    AL = mybir.AluOpType
    AF = mybir.ActivationFunctionType

    # ================= LSTM x-projection (layer 1), all timesteps =========
    # f32r matmuls: 1 cycle/row at >=256 output cols vs 4 for fp32.
    # Time-major chunks so the recurrence can start after the first chunk;
    # copies on DVE only (no Act Identity -> no act-table switch).
    Zx1 = lstm.tile([128, 4, T * BL], F32, tag="Zx1")
    CW = 256
    nzc = (T * BL + CW - 1) // CW
    with tc.high_priority(offset=-1000000):
        for ci in range(nzc):
            c0, c1 = ci * CW, min((ci + 1) * CW, T * BL)
            for g in range(4):
                pz = ps_zx.tile([128, CW], F32, tag="pzx")
                nc.tensor.matmul(pz[:, :c1 - c0],
                                 k1p[:, g * 128:(g + 1) * 128],
                                 seqT[:, c0:c1])
                nc.vector.tensor_scalar_add(Zx1[:, g, c0:c1],
                                            pz[:, :c1 - c0],
                                            b1p[:, g:g + 1])

    # ================= prologue work units ================================
    # Each unit is a closure emitting a small instruction group; the LSTM
    # loop below pops a few per step so they fill idle engine time.
    units = []

    S1 = stats.tile([128, NBC], F32, tag="S1")
    S2 = stats.tile([128, NBC], F32, tag="S2")
    S3 = stats.tile([128, NBC], F32, tag="S3")
    S4 = stats.tile([128, NBC], F32, tag="S4")
    S1h = stats.tile([128, NBC], F32, tag="S1h")
    S2h = stats.tile([128, NBC], F32, tag="S2h")
    St = stats.tile([128, NBC], F32, tag="St")
    MEAN = stats.tile([128, NBC], F32, tag="MEAN")

    def stats_units(b, nk):
        col = b * NCH + nk
        box = {}

        def u_dma(b=b, nk=nk, box=box):
            xt = xpool.tile([128, H], F32, tag="xt", name="xt")
            nc.sync.dma_start(xt[:], d["x0t"][b, nk * 128:(nk + 1) * 128, :])
            box["xt"] = xt

        def u1(col=col, box=box):
            xt = box["xt"]
            sc2 = scr.tile([128, H], F32, tag="c2sc", name="sc2")
            sc3 = scr.tile([128, H], F32, tag="c3sc", name="sc3")
            # x^2 (+ sum), x * tc slope (+ sum)
            nc.vector.scalar_tensor_tensor(sc2[:], xt[:], 1.0, xt[:],
                                           AL.bypass, AL.mult,
                                           accum_out=S2[:, col:col + 1])
            nc.vector.scalar_tensor_tensor(sc3[:], xt[:], 1.0, tcb[:],
                                           AL.bypass, AL.mult,
                                           accum_out=St[:, col:col + 1])
            box["sc2"] = sc2

        def u2(col=col, box=box):
            xt, sc2 = box["xt"], box["sc2"]
            sc3 = scr.tile([128, H], F32, tag="c3sc", name="sc3b")
            # x^3 (+ sum), x^4 (+ sum)
            nc.vector.scalar_tensor_tensor(sc3[:], sc2[:], 1.0, xt[:],
                                           AL.bypass, AL.mult,
                                           accum_out=S3[:, col:col + 1])
            nc.vector.scalar_tensor_tensor(sc3[:], sc2[:], 1.0, sc2[:],
                                           AL.bypass, AL.mult,
                                           accum_out=S4[:, col:col + 1])

        def u3(col=col, box=box):
            xt, sc2 = box["xt"], box["sc2"]
            nc.vector.reduce_sum(S1[:, col:col + 1], xt[:],
                                 axis=mybir.AxisListType.X)
            nc.vector.reduce_sum(S1h[:, col:col + 1], xt[:, HH:],
                                 axis=mybir.AxisListType.X)
            nc.vector.reduce_sum(S2h[:, col:col + 1], sc2[:, HH:],
                                 axis=mybir.AxisListType.X)

        return [u_dma, u1, u2, u3]

    tile_units = [stats_units(b, nk) for b in range(BL) for nk in range(NCH)]
    # dma two tiles ahead of their compute
    units += [tile_units[0][0], tile_units[1][0]]
    for i, tu in enumerate(tile_units):
        units += tu[1:]
        if i + 2 < len(tile_units):
            units.append(tile_units[i + 2][0])

    # ---- combine into NF (mean, mean_half, std, std_half, skew, kurt, slope)
    NF = stats.tile([128, 7, NBC], F32, tag="NF")
    w = stats.tile([128, 12, NBC], F32, tag="wrk")

    def comb1():
        nc.vector.tensor_scalar_mul(MEAN[:], S1[:], _K168)          # mu
        nc.vector.tensor_copy(NF[:, 0, :], MEAN[:])
        nc.vector.tensor_scalar_mul(NF[:, 1, :], S1h[:], _K84)      # mu_h
        nc.vector.tensor_tensor(w[:, 0, :], MEAN[:], MEAN[:], AL.mult)  # q
        nc.vector.tensor_scalar_mul(w[:, 1, :], S2[:], _K168)       # r2
        nc.vector.tensor_scalar_mul(w[:, 2, :], S3[:], _K168)       # r3
        nc.vector.tensor_scalar_mul(w[:, 3, :], S4[:], _K168)       # r4
        nc.vector.tensor_tensor(w[:, 4, :], w[:, 1, :], w[:, 0, :],
                                AL.subtract)                        # m2
        nc.vector.reciprocal(w[:, 5, :], w[:, 4, :])                # 1/m2

    def comb2():
        nc.gpsimd.tensor_tensor(w[:, 6, :], NF[:, 1, :], NF[:, 1, :],
                                AL.mult)                            # mu_h^2
        nc.vector.scalar_tensor_tensor(w[:, 7, :], S2h[:], _K84,
                                       w[:, 6, :], AL.mult,
                                       AL.subtract)                 # var_h
        nc.vector.tensor_scalar_mul(w[:, 8, :], w[:, 0, :], 2.0)    # 2q
        nc.vector.scalar_tensor_tensor(w[:, 8, :], w[:, 1, :], 3.0,
                                       w[:, 8, :], AL.mult,
                                       AL.subtract)                 # 3r2-2q
        nc.gpsimd.tensor_tensor(w[:, 8, :], w[:, 8, :], MEAN[:], AL.mult)
        nc.vector.tensor_tensor(w[:, 8, :], w[:, 2, :], w[:, 8, :],
                                AL.subtract)                        # m3

    def _rsqrt(zs, v, r, scr):
        """zs = 1/sqrt(v) via mixed-basis seed + 2 Newton iterations.
        All plain DVE ops -- avoids the Act Sqrt table (the whole kernel
        then only ever loads the Sigmoid act table)."""
        C0, C1, C2, C3 = 0.53984165, -0.04446299, 0.53637199, -0.04129085
        nc.vector.tensor_scalar(scr, r, C3, C2, AL.mult, AL.add)
        nc.gpsimd.tensor_tensor(scr, scr, r, AL.mult)
        nc.vector.tensor_scalar(zs, v, C1, C0, AL.mult, AL.add)
        nc.vector.tensor_tensor(zs, zs, scr, AL.add)
        for _ in range(2):
            nc.gpsimd.tensor_tensor(scr, zs, zs, AL.mult)
            nc.gpsimd.tensor_tensor(scr, scr, v, AL.mult)
            nc.vector.tensor_scalar(scr, scr, -0.5, 1.5, AL.mult, AL.add)
            nc.vector.tensor_tensor(zs, zs, scr, AL.mult)

    def comb3():
        _rsqrt(w[:, 9, :], w[:, 4, :], w[:, 5, :], w[:, 6, :])
        nc.vector.tensor_tensor(NF[:, 2, :], w[:, 4, :], w[:, 9, :],
                                AL.mult)                            # std
        nc.vector.reciprocal(w[:, 10, :], w[:, 7, :])
        _rsqrt(w[:, 11, :], w[:, 7, :], w[:, 10, :], w[:, 6, :])
        nc.vector.tensor_tensor(NF[:, 3, :], w[:, 7, :], w[:, 11, :],
                                AL.mult)                            # std_h
        # skew = m3 * m2^-1.5
        nc.gpsimd.tensor_tensor(w[:, 9, :], w[:, 9, :], w[:, 5, :],
                                AL.mult)                            # m2^-1.5
        nc.vector.tensor_tensor(NF[:, 4, :], w[:, 8, :], w[:, 9, :],
                                AL.mult)
        # m4 = r4 - 4*mu*r3 + 6*q*r2 - 3*q^2
        nc.vector.scalar_tensor_tensor(w[:, 6, :], w[:, 2, :], 4.0,
                                       MEAN[:], AL.mult, AL.mult)   # 4 mu r3
        nc.vector.scalar_tensor_tensor(w[:, 7, :], w[:, 1, :], 6.0,
                                       w[:, 0, :], AL.mult, AL.mult)  # 6 q r2
        nc.gpsimd.tensor_tensor(w[:, 9, :], w[:, 0, :], w[:, 0, :],
                                AL.mult)                            # q^2
        nc.vector.tensor_tensor(w[:, 6, :], w[:, 3, :], w[:, 6, :],
                                AL.subtract)                        # r4 - 4mur3
        nc.vector.scalar_tensor_tensor(w[:, 7, :], w[:, 9, :], -3.0,
                                       w[:, 7, :], AL.mult, AL.add)
        nc.vector.tensor_tensor(w[:, 6, :], w[:, 6, :], w[:, 7, :],
                                AL.add)                             # m4
        # kurt = m4 / m2^2 - 3
        nc.gpsimd.tensor_tensor(w[:, 9, :], w[:, 5, :], w[:, 5, :],
                                AL.mult)
        nc.vector.tensor_tensor(w[:, 6, :], w[:, 6, :], w[:, 9, :],
                                AL.mult)
        nc.vector.tensor_scalar_add(NF[:, 5, :], w[:, 6, :], -3.0)
        nc.vector.tensor_scalar_mul(NF[:, 6, :], St[:], _KSLOPE)
        if dbg is not None and "nf" in dbg:
            nc.sync.dma_start(dbg["nf"][:], NF[:])

    units_a = units
    units_c = [comb1, comb2, comb3]
    units = []

    # ---- GCN ----
    NFT = gcn.tile([7, NBC * 128], F32, tag="NFT")
    T1 = gcn.tile([128, NBC, 32], F32, tag="T1")
    H1 = gcn.tile([128, NBC, 32], F32, tag="H1")
    H1T = gcn.tile([32, NBC * 128], F32, tag="H1T")
    T2 = gcn.tile([128, NBC, 16], F32, tag="T2")
    G = gcn.tile([128, NBC, 16], F32, tag="G")

    def u_nft(q):
        def f(q=q):
            pt = ps_a.tile([7, 512], F32, tag="a", name="pt")
            for j in range(4):
                nc.tensor.transpose(pt[:, j * 128:(j + 1) * 128],
                                    NF[:, :, q * 4 + j], I128[:])
            nc.vector.tensor_copy(NFT[:, q * 512:(q + 1) * 512], pt[:])
        return f

    def u_t1(bc):
        def f(bc=bc):
            for k in range(2):
                pt = ps_a.tile([128, 32], F32, tag="a", name="pt")
                nc.tensor.matmul(pt[:], NFT[:, (bc + k) * 128:
                                             (bc + k + 1) * 128], w1[:])
                nc.vector.tensor_copy(T1[:, bc + k, :], pt[:])
        return f

    def u_h1(b, nk):
        def f(b=b, nk=nk):
            ph = ps_a.tile([128, 32], F32, tag="a", name="ph")
            for mc in range(NCH):
                nc.tensor.matmul(ph[:], adjT[:, mc * N + nk * 128:
                                              mc * N + (nk + 1) * 128],
                                 T1[:, b * NCH + mc, :],
                                 start=(mc == 0), stop=False)
            nc.tensor.matmul(ph[:], onesr[:1, :], b1row[:], start=False,
                             stop=True)
            nc.vector.tensor_scalar_max(H1[:, b * NCH + nk, :], ph[:], 0.0)
        return f

    def u_h1t(q):
        def f(q=q):
            pt = ps_a.tile([32, 512], F32, tag="a", name="pt")
            for j in range(4):
                nc.tensor.transpose(pt[:, j * 128:(j + 1) * 128],
                                    H1[:, q * 4 + j, :], I128[:])
            nc.vector.tensor_copy(H1T[:, q * 512:(q + 1) * 512], pt[:])
        return f

    def u_t2(bc):
        def f(bc=bc):
            for k in range(2):
                pt = ps_a.tile([128, 16], F32, tag="a", name="pt")
                nc.tensor.matmul(pt[:], H1T[:, (bc + k) * 128:
                                             (bc + k + 1) * 128], w2[:])
                nc.vector.tensor_copy(T2[:, bc + k, :], pt[:])
        return f

    def u_g(b, nk):
        def f(b=b, nk=nk):
            ph = ps_a.tile([128, 16], F32, tag="a", name="ph")
            for mc in range(NCH):
                nc.tensor.matmul(ph[:], adjT[:, mc * N + nk * 128:
                                              mc * N + (nk + 1) * 128],
                                 T2[:, b * NCH + mc, :],
                                 start=(mc == 0), stop=False)
            nc.tensor.matmul(ph[:], onesr[:1, :], b2row[:], start=False,
                             stop=True)
            nc.vector.tensor_scalar_max(G[:, b * NCH + nk, :], ph[:], 0.0)
        return f

    units += [u_nft(q) for q in range(NBC // 4)]
    units += [u_t1(bc) for bc in range(0, NBC, 2)]
    units += [u_h1(b, nk) for b in range(BL) for nk in range(NCH)]
    units += [u_h1t(q) for q in range(NBC // 4)]
    units += [u_t2(bc) for bc in range(0, NBC, 2)]
    units += [u_g(b, nk) for b in range(BL) for nk in range(NCH)]

    def u_dbg_g():
        if dbg is not None and "g" in dbg:
            nc.sync.dma_start(dbg["g"][:], G[:])
    units.append(u_dbg_g)

    # ---- Conv1D head ----
    # c1[o, 16b+l] = sum_d sum_n g[b, n, l+d-1] * w1c[d, n, o]
    pc1 = ps_a.tile([4, 16 * BL], F32, tag="pc1", bufs=1)
    c1sb = gcn.tile([4, 16 * BL], F32, tag="c1sb")
    GH = gcn.tile([4, 16 * BL], F32, tag="GH")   # per b: [c2(8) | p'(8)]
    pv = GH[:].rearrange("p (b h l) -> p b h l", b=BL, h=2)
    c1v = c1sb[:].rearrange("p (b l e) -> p b l e", b=BL, e=2)

    def u_conv1(b):
        def f(b=b):
            first = b == 0
            for dd in (1, 0, 2):  # full-width shift first (start coverage)
                lo, hi = max(0, 1 - dd), min(16, 17 - dd)
                for nk in range(NCH):
                    nc.tensor.matmul(
                        pc1[:, 16 * b + lo:16 * b + hi],
                        wc1[:, dd, nk, :],
                        G[:, b * NCH + nk, lo + dd - 1:hi + dd - 1],
                        start=first, stop=(b == BL - 1 and dd == 2
                                           and nk == NCH - 1))
                    first = False
        return f

    units += [u_conv1(b) for b in range(BL)]

    def u_conv2():
        nc.vector.tensor_copy(c1sb[:], pc1[:])
        # p' = c1e + c1o + 2*b_conv1  (scale 0.5 folded into w2ch/Whead)
        nc.vector.scalar_tensor_tensor(pv[:, :, 1, :], c1v[:, :, :, 0],
                                       b1c2[:], c1v[:, :, :, 1],
                                       AL.add, AL.add)

    def u_conv3():
        # c2 = conv2(p') + b_conv2
        pc2 = ps_a.tile([4, 8 * BL], F32, tag="a", name="pc2")
        for b in range(BL):
            first = True
            for dd in (1, 0, 2):
                lo, hi = max(0, 1 - dd), min(8, 9 - dd)
                nc.tensor.matmul(pc2[:, 8 * b + lo:8 * b + hi],
                                 w2ch[:, dd, :],
                                 pv[:, b, 1, lo + dd - 1:hi + dd - 1],
                                 start=first, stop=(dd == 2))
                first = False
        pc2v = pc2[:].rearrange("p (b l) -> p b l", b=BL)
        nc.vector.tensor_scalar_add(pv[:, :, 0, :], pc2v[:], b2c[:])

    featT = gcn.tile([16, 4 * BL], F32, tag="featT")

    def u_conv4():
        # transpose per b: (4, 16) -> (16, 4); featT cols = 4b + o
        pft = ps_a.tile([16, 4 * BL], F32, tag="a", name="pft")
        for b in range(BL):
            nc.tensor.transpose(pft[:, 4 * b:4 * b + 4],
                                GH[:, 16 * b:16 * (b + 1)], I128[:4, :4])
        nc.vector.tensor_copy(featT[:], pft[:])

    units += [u_conv2, u_conv3, u_conv4]
    units_b = units

    # ================= LSTM recurrence ====================================
    cc = lstm.tile([128, 2, GB], F32, tag="cc", name="cc")
    ascr = lstm.tile([128, 1], F32, tag="ascr", name="ascr")
    nc.vector.memset(cc[:], 0.0)

    Zx1v = Zx1[:].rearrange("p g (t b) -> p g t b", b=BL)

    st = {"o": None, "m": None, "gt": None}

    def emit_P1(t):
        pz = ps_z.tile([128, 2, 4 * GB], F32, tag="pz", name="pz")
        gt = zpool.tile([128, 2, 4 * GB], F32, tag="gt", name="gt", bufs=3)
        do1, do2 = t < T, t > 0
        op, mp = st["o"], st["m"]
        ops = []   # (dst, lhsT, rhs): o-part first, m-part last
        if do1:
            ops.append((pz[:, 0, :], I128[:], Zx1v[:, :, t, :]))
            if t > 0:
                for g in range(4):
                    ops.append((pz[:, 0, g * GB:(g + 1) * GB],
                                rkO1[:, g * 128:(g + 1) * 128], op[:, 0, :]))
        if do2:
            ops.append((pz[:, 1, :], b2p4[:], sel4[:]))
            for g in range(4):
                ops.append((pz[:, 1, g * GB:(g + 1) * GB],
                            kO2[:, g * 128:(g + 1) * 128], op[:, 0, :]))
            if t > 1:
                for g in range(4):
                    ops.append((pz[:, 1, g * GB:(g + 1) * GB],
                                rkO2[:, g * 128:(g + 1) * 128], op[:, 1, :]))
        if do1 and t > 0:
            for g in range(4):
                ops.append((pz[:, 0, g * GB:(g + 1) * GB],
                            rkM1[:, g * 128:(g + 1) * 128], mp[:, 0, :]))
        if do2 and t > 0:
            for g in range(4):
                ops.append((pz[:, 1, g * GB:(g + 1) * GB],
                            kM2[:, g * 128:(g + 1) * 128], mp[:, 0, :]))
            if t > 1:
                for g in range(4):
                    ops.append((pz[:, 1, g * GB:(g + 1) * GB],
                                rkM2[:, g * 128:(g + 1) * 128], mp[:, 1, :]))
        for j, (dst, lhsT, rhs) in enumerate(ops):
            nc.tensor.matmul(dst, lhsT, rhs, start=(j == 0),
                             stop=(j == len(ops) - 1))

        l0, l1 = (0 if do1 else 1), (2 if do2 else 1)
        nc.scalar.activation(gt[:, l0:l1, :], pz[:, l0:l1, :], AF.Sigmoid)
        st["gt"] = gt
        st["o"] = gt[:, :, 2 * GB:3 * GB]

    def emit_P2(t):
        gt = st["gt"]
        do1, do2 = t < T, t > 0
        l0, l1 = (0 if do1 else 1), (2 if do2 else 1)
        a = zpool.tile([128, 2, GB], F32, tag="a", name="a", bufs=2)
        v = zpool.tile([128, 2, GB], F32, tag="v", name="v", bufs=2)
        s2 = zpool.tile([128, 2, GB], F32, tag="s2", name="s2", bufs=2)
        m = zpool.tile([128, 2, GB], F32, tag="m", name="m", bufs=3)
        nc.vector.tensor_tensor(v[:, l0:l1, :], gt[:, l0:l1, GB:2 * GB],
                                cc[:, l0:l1, :], AL.mult)
        nc.vector.affine_mul_reduce(a[:, l0:l1, :], ascr[:],
                                    gt[:, l0:l1, 3 * GB:],
                                    gt[:, l0:l1, 0:GB], 2.0, -1.0)
        nc.vector.affine_then_add(cc[:, l0:l1, :], v[:, l0:l1, :],
                                  a[:, l0:l1, :], 1.0, 0.0)
        # per-layer s2/m: only L1's pair sits on the recurrence chain
        if do1:
            nc.scalar.activation(s2[:, 0:1, :], cc[:, 0:1, :],
                                 AF.Sigmoid, scale=2.0)
            nc.vector.affine_mul_reduce(m[:, 0:1, :], ascr[:],
                                        s2[:, 0:1, :],
                                        gt[:, 0:1, 2 * GB:3 * GB], 1.0, 0.0)
        if do2:
            nc.scalar.activation(s2[:, 1:2, :], cc[:, 1:2, :],
                                 AF.Sigmoid, scale=2.0)
            nc.vector.affine_mul_reduce(m[:, 1:2, :], ascr[:],
                                        s2[:, 1:2, :],
                                        gt[:, 1:2, 2 * GB:3 * GB], 1.0, 0.0)
        st["m"] = m

    UW_COMB = float(os.environ.get("UW_COMB", "0.095"))
    UW_GCN = float(os.environ.get("UW_GCN", "0.102"))
    UW_STEP = float(os.environ.get("UW_STEP", "0.0014"))
    ua, ub = units_a, units_b
    ia = ib = 0
    for t in range(T + 1):
        emit_P1(t)
        if dbg is not None and t == 0 and "gt0" in dbg:
            nc.sync.dma_start(dbg["gt0"][:], st["gt"][:])
        if dbg is not None and t == 1 and "gt1" in dbg:
            nc.sync.dma_start(dbg["gt1"][:], st["gt"][:])
        emit_P2(t)
        if dbg is not None and t == 0 and "cc0" in dbg:
            nc.sync.dma_start(dbg["cc0"][:], cc[:])
        if t >= 1:
            for _ in range(3):
                if ia < len(ua):
                    with tc.high_priority(offset=-1000000):
                        ua[ia]()
                    ia += 1
        if t == 46:
            for j, u in enumerate(units_c):
                with tc.tile_wait_until(UW_COMB + 0.002 * j), \
                        tc.high_priority(offset=-1000000):
                    u()
        if t >= 48:
            for _ in range(3):
                if ib < len(ub):
                    with tc.tile_wait_until(UW_GCN + UW_STEP * ib), \
                            tc.high_priority(offset=-1000000):
                        ub[ib]()
                    ib += 1
    while ia < len(ua):
        ua[ia]()
        ia += 1
    while ib < len(ub):
        ub[ib]()
        ib += 1

    # ================= output head ========================================
    hfin = gcn.tile([128, BL], F32, tag="hfin")
    nc.vector.scalar_tensor_tensor(hfin[:], st["m"][:, 1, :], 2.0,
                                   st["o"][:, 1, :], AL.mult, AL.subtract)
    if dbg is not None and "hfin" in dbg:
        nc.sync.dma_start(dbg["hfin"][:], hfin[:])

    po = ps_a.tile([BL, P], F32, tag="po", bufs=1)
    nc.tensor.matmul(po[:], onesr[:1, :BL], b_out_row[:], start=True,
                     stop=False)
    fv = featT[:].rearrange("p (b o) -> p b o", o=4)
    for o in range(4):
        nc.tensor.matmul(po[:], fv[:, :, o], Whead[:, o, :], start=False,
                         stop=False)
    nc.tensor.matmul(po[:], hfin[:], Wlstm[:], start=False, stop=True)
    osb = gcn.tile([BL, P], F32, tag="osb")
    nc.vector.tensor_copy(osb[:], po[:])
    nc.sync.dma_start(out[:], osb[:])


def _build(dbg_names=()):
    key = tuple(sorted(dbg_names))
    if key in _CACHE:
        return _CACHE[key]
    nc = bacc.Bacc("TRN2", target_bir_lowering=False, debug=False,
                   num_devices=N_CORES)
    with tile.TileContext(nc) as tc:
        with ExitStack() as ctx:
            dbg = {}
            if "nf" in key:
                dbg["nf"] = nc.dram_tensor("dbg_nf", [128, 7, NBC], F32,
                                           kind="ExternalOutput").ap()
            if "g" in key:
                dbg["g"] = nc.dram_tensor("dbg_g", [128, NBC, 16], F32,
                                          kind="ExternalOutput").ap()
            if "hfin" in key:
                dbg["hfin"] = nc.dram_tensor("dbg_hfin", [128, BL], F32,
                                             kind="ExternalOutput").ap()
            for nm, shp in [("gt0", [128, 2, 4 * GB]),
                            ("gt1", [128, 2, 4 * GB]),
                            ("cc0", [128, 2, GB])]:
                if nm in key:
                    dbg[nm] = nc.dram_tensor("dbg_" + nm, shp, F32,
                                             kind="ExternalOutput").ap()
            _emit_kernel(nc, tc, ctx, dbg=dbg or None)
    nc.compile()
    _CACHE[key] = nc
    return nc


def _prep(inputs):
    x0 = np.ascontiguousarray(inputs["inputs"][..., 0])          # (B, H, N)
    x0t = np.ascontiguousarray(x0.transpose(0, 2, 1))            # (B, N, H)
    seq = inputs["inputs"][:, :, 0, :]                           # (B, H, F)
    adjT = np.ascontiguousarray(inputs["adj"].T)
    tc_vec = (np.arange(H, dtype=np.float32) - (H - 1) / 2.0)
    tc_bc = np.broadcast_to(tc_vec, (128, H)).copy()
    I128 = np.eye(128, dtype=np.float32)
    ones_row = np.ones((1, 128), np.float32)

    perm = np.concatenate([np.arange(0, 128), np.arange(128, 256),
                           np.arange(384, 512), np.arange(256, 384)])
    gs = np.ones(512, np.float32)
    gs[384:] = 2.0            # g-gate pre-scale: tanh(x) = 2*sig(2x) - 1
    k1p = inputs["k_lstm1"][:, perm] * gs
    rk1s = inputs["rk_lstm1"][:, perm] * gs
    b1p = (inputs["b_lstm1"][perm] * gs).reshape(4, 128).T
    k2s = inputs["k_lstm2"][:, perm] * gs
    rk2s = inputs["rk_lstm2"][:, perm] * gs
    b2p4 = (inputs["b_lstm2"][perm] * gs).reshape(4, 128)
    sel4 = np.zeros((4, 4 * GB), np.float32)
    for g in range(4):
        sel4[g, g * GB:(g + 1) * GB] = 1.0

    w_out = inputs["w_out"]
    Whead = np.zeros((16, 4, P), np.float32)
    for o in range(4):
        for l in range(8):
            Whead[l, o, :] = w_out[o * 8 + l, :]                 # c2 rows
            Whead[8 + l, o, :] = 0.5 * w_out[32 + o * 8 + l, :]  # p rows
    Wlstm = w_out[64:192, :]

    com = {
        "adjT": adjT, "tc_bc": tc_bc, "I128": I128, "ones_row": ones_row,
        "w1": inputs["w_gcn1"], "b1row": inputs["b_gcn1"][None, :],
        "w2": inputs["w_gcn2"], "b2row": inputs["b_gcn2"][None, :],
        "w1c": inputs["w_conv1"], "b1c2": 2.0 * inputs["b_conv1"][:, None],
        "w2ch": 0.5 * np.asarray(inputs["w_conv2"]).transpose(1, 0, 2),
        "b2c": inputs["b_conv2"][:, None],
        "k1p": k1p, "b1p": b1p,
        "rkM1": 2.0 * rk1s, "rkO1": -rk1s,
        "kM2": 2.0 * k2s, "kO2": -k2s,
        "rkM2": 2.0 * rk2s, "rkO2": -rk2s,
        "b2p4": b2p4, "sel4": sel4, "Whead": Whead, "Wlstm": Wlstm,
        "b_out_row": inputs["b_out"][None, :],
    }
    com = {k: np.ascontiguousarray(v, dtype=np.float32)
           for k, v in com.items()}

    in_maps = []
    for c in range(N_CORES):
        bs = slice(c * BL, (c + 1) * BL)
        m = dict(com)
        m["x0t"] = np.ascontiguousarray(x0t[bs])
        m["seqT"] = np.ascontiguousarray(
            np.asarray(seq[bs]).transpose(2, 1, 0).reshape(F, T * BL))
        in_maps.append(m)
    return in_maps


def kernel(**inputs):
    nc = _build()
    in_maps = _prep(inputs)
    res = run_bass_kernel_spmd(nc, in_maps, list(range(N_CORES)))
    return np.concatenate([res.results[c]["out"] for c in range(N_CORES)],
                          axis=0)
